# revision 20
# baseline (speedup 1.0000x reference)
"""ARD RBF kernel matrix on 8 TRN2 NeuronCores.

out[n, m] = exp(log_outputscale) * exp(-0.5 * sum_d ((x[n,d] - y[m,d]) / l_d)^2)
with l = exp(log_lengthscale).

Per core (rows of x sharded 8-ways), with invl2[d] = exp(-2*log_lengthscale[d]):
the lengthscale is folded into the X side only —
    cross[n, m] = sum_d (x[n,d]*invl2[d]) * y[m,d]
    y2[m] = sum_d y[m,d]^2 * (-0.5*invl2[d])   (matmul, weights -0.5*invl2)
    x2[n] = sum_d x[n,d]^2 * (-0.5*invl2[d]) + log_outputscale
    out[n, m] = Exp(cross + y2 + x2)
so y needs NO elementwise scaling: one DVE f32->f32r rounding copy and
one DVE square per chunk, and the y2 row rides the matmul. One K=97
fp32r matmul per output tile: lhsT = [x*invl2; 1; 0*31; 1],
rhs = [y; y2hi; junk*31; y2lo]. The zero rows in lhsT kill the junk
rows; y2 is split hi+lo (fp32r residual pair, both at DVE-writable
partitions 64/96) so its rounding error stays ~1e-6; x2 + log_os ride
the ACT bias in full f32. exp runs as one ScalarE pass per [128, 2048]
PSUM chunk, written as bf16 (8-bit exponent covers the e^-60-scale
tail; ~0.4% rounding) and upcast to f32 on the host — halves the
output HBM traffic. Measured HW rel err ~1e-3.

Inputs are staged host-side in transposed layout ([D, points]) so the
contraction dim lands on SBUF partitions with no on-device transposes.

Schedule: input DMAs first on the SP ring (x, then y in 4 chunks);
DVE preps x_aug while y chunk 0 lands; per y chunk DVE rounds + squares
and PE reduces y2 (borrowing row 0 of main-pool PSUM tiles — no
separate PSUM pool, no pool barrier); then the 8x16 main matmul /
exp sweep streams row tiles out as 1 MiB bf16 DMAs (first row tile
ships in 0.5 MiB chunks to start the output stream early).

build_nc(repeat=R) emits the whole computation R times into one NEFF
(reps serialized by buffer reuse) — used by test.py to measure the
per-iteration device makespan as a slope, amortizing dispatch overhead.
main_mm/act/outdma=False build ablation variants for differential
timing only (wrong results).
"""

import numpy as np

import concourse.bass as bass
import concourse.mybir as mybir
import concourse.tile as tile
from concourse import bacc
from concourse.bass_utils import run_bass_kernel_spmd

N_CORES = 8
N, M, D = 8192, 8192, 64
NSH = N // N_CORES  # 1024 x-rows per core

F32 = mybir.dt.float32
F32R = mybir.dt.float32r
BF16 = mybir.dt.bfloat16
AF = mybir.ActivationFunctionType


def build_nc(nsh=NSH, m=M, d=D, use_f32r=True, n_chunk=2048, out_bf16=True,
             repeat=1, main_mm=True, act=True, outdma=True, dma_rings=2,
             mm_x=1, act_x=1, skip_yprep=False):
    """Per-core Bass graph. SPMD: same graph on all 8 cores."""
    nc = bacc.Bacc("TRN2", target_bir_lowering=False)

    odt = BF16 if out_bf16 else F32

    xT = nc.dram_tensor("xT", [d, nsh], F32, kind="ExternalInput")
    yT = nc.dram_tensor("yT", [d, m], F32, kind="ExternalInput")
    lls = nc.dram_tensor("log_lengthscale", [d], F32, kind="ExternalInput")
    los = nc.dram_tensor("log_outputscale", [1], F32, kind="ExternalInput")
    out = nc.dram_tensor("out", [nsh, m], odt, kind="ExternalOutput")

    n_tiles = nsh // 128          # x tiles (output partition dim)
    mm_n = 512                    # moving free dim per matmul (one PSUM bank)
    n_sub = n_chunk // mm_n       # matmuls per ACT chunk
    mc = m // n_chunk             # y chunks
    K = 97                        # contraction: 64 data + y2hi@64 + 31 dead + y2lo@96
    ot_w = m // 2                 # output tile width (1 MiB bf16 DMAs)

    def mmi(ap):  # matmul input view
        return ap.bitcast(F32R) if use_f32r else ap

    def mmo(ap):  # rounded-writer output view (BIR fp32r-producer rule)
        return ap.bitcast(F32R) if use_f32r else ap

    # output DMA queue rotation (SP always; optionally ACT HWDGE / SWDGE)
    rings = [nc.sync, nc.scalar, nc.gpsimd][:dma_rings]

    with tile.TileContext(nc) as tc:
        with (
            tc.tile_pool(name="const", bufs=1) as cpool,
            tc.tile_pool(name="xb", bufs=2) as xb,
            tc.tile_pool(name="yraw", bufs=2) as yrp,
            tc.tile_pool(name="ysqp", bufs=2) as ysp,
            tc.tile_pool(name="outp", bufs=4) as opool,
            tc.tile_pool(name="mainps", bufs=2, space="PSUM") as mp,
        ):
            # ---- hyperparameters and constants (once; reps reuse) ----
            lls_sb = cpool.tile([d, 1], F32, tag="lls")
            nc.sync.dma_start(out=lls_sb[:, :], in_=lls[:].rearrange("(d o) -> d o", o=1))
            los_sb = cpool.tile([1, 1], F32, tag="los")
            nc.sync.dma_start(out=los_sb[:, :], in_=los[:].rearrange("(a o) -> a o", o=1))

            invl2 = cpool.tile([d, 1], F32, tag="invl2")  # exp(-2*lls)
            nc.scalar.activation(invl2[:, :], lls_sb[:, :], AF.Exp, scale=-2.0)
            l2 = cpool.tile([d, 1], F32, tag="l2")  # exp(+2*lls)
            nc.scalar.activation(l2[:, :], lls_sb[:, :], AF.Exp, scale=2.0)
            neghalf_f = cpool.tile([d, 1], F32, tag="neghalf_f")
            nc.vector.memset(neghalf_f[:, :], -0.5)
            # y2 reduce weights -0.5*invl2 (ysq holds rounded raw y squares);
            # x2 reduce weights -0.5*exp(2*lls) (xsq holds rounded x*invl2
            # squares) — together they keep sq_dist a perfect square:
            # c + x2 + y2 = -0.5*sum((a/l - y*invl)^2) for a = (x*invl2)_r
            nhi2 = cpool.tile([d, 1], F32, tag="nhi2")
            nc.vector.tensor_mul(mmo(nhi2[:, :]), invl2[:, :], neghalf_f[:, :])
            nhx2 = cpool.tile([d, 1], F32, tag="nhx2")
            nc.vector.tensor_mul(mmo(nhx2[:, :]), l2[:, :], neghalf_f[:, :])
            ones11 = cpool.tile([1, 1], F32, tag="ones11")
            nc.vector.memset(ones11[:, :], 1.0)

            # lhsT aux rows [33, nsh]: 1 at row 64, zeros 65..95, 1 at 96
            aux_rows = cpool.tile([33, nsh], F32, tag="aux_rows")
            nc.vector.memset(aux_rows[:, :], 0.0)
            nc.vector.memset(aux_rows[0:1, :], 1.0)
            nc.vector.memset(aux_rows[32:33, :], 1.0)

            x2row = cpool.tile([1, nsh], F32, tag="x2row")
            bias_sb = cpool.tile([128, n_tiles], F32, tag="bias")

            # y_aug allocated once: rows 64:96 zeroed once so the dead rows
            # 65..95 stay finite forever (0 * junk would NaN the PSUM sum).
            y_aug = cpool.tile([K, m], F32, tag="y_aug")
            nc.vector.memset(y_aug[64:96, :], 0.0)

            for _rep in range(repeat):
                # ---- input DMAs on the SWDGE (gpsimd) queue: never queue
                # behind the previous rep's output DMAs on the SP ring ----
                x_raw = xb.tile([d, nsh], F32, tag="x_raw")
                nc.gpsimd.dma_start(out=x_raw[:, :], in_=xT[:, :])
                y_raws = []
                for jc in range(mc):
                    y_raw = yrp.tile([d, n_chunk], F32, tag="y_raw")
                    nc.gpsimd.dma_start(
                        out=y_raw[:, :], in_=yT[:, jc * n_chunk : (jc + 1) * n_chunk]
                    )
                    y_raws.append(y_raw)

                # ---- x side (DVE): x_aug = [x*invl2; 1; 0...; 1], xsq = x^2 ----
                x_aug = xb.tile([K, nsh], F32, tag="x_aug")
                nc.vector.tensor_scalar_mul(mmo(x_aug[0:d, :]), x_raw[:, :], invl2[:, :])
                # square the ROUNDED operands (not raw) so sq_dist keeps its
                # perfect-square structure in the rounded values
                xsq = xb.tile([d, nsh], F32, tag="xsq")
                nc.vector.tensor_mul(mmo(xsq[:, :]), x_aug[0:d, :], x_aug[0:d, :])
                nc.vector.tensor_copy(mmo(x_aug[64:K, :]), aux_rows[:, :])

                # ---- x2 row: -0.5*sum x^2*exp(2lls) + log_os (one PSUM tile,
                # one ACT identity) ----
                psx = mp.tile([128, n_chunk], F32, tag="mm")
                for j0 in range(0, nsh, mm_n):
                    nc.tensor.matmul(
                        psx[0:1, j0 : j0 + mm_n], mmi(nhx2[:, :]),
                        mmi(xsq[:, j0 : j0 + mm_n]),
                        start=True, stop=True,
                    )
                nc.scalar.activation(
                    x2row[:, :], psx[0:1, 0:nsh], AF.Identity, bias=los_sb[:, :],
                )

                # ---- y prep, split so the PSUM-borrowing half (psy matmuls
                # + hi/lo rows) can be emitted AFTER a main chunk's 8 tiles:
                # its borrowed buffer then has a whole ACT-chunk window
                # (~14us) before the next main tile WARs on it. ----
                def y_prep_pre(jc):
                    # rounding copy + square: SBUF only, no PSUM deps
                    if skip_yprep:
                        return None
                    slc = slice(jc * n_chunk, (jc + 1) * n_chunk)
                    y_raw = y_raws[jc]
                    nc.vector.tensor_copy(mmo(y_aug[0:d, slc]), y_raw[:, :])
                    ysq = ysp.tile([d, n_chunk], F32, tag="ysq")
                    nc.vector.tensor_mul(
                        mmo(ysq[:, :]), y_aug[0:d, slc], y_aug[0:d, slc]
                    )
                    return ysq

                def y_prep_post(jc, ysq):
                    # y2 reduce into a borrowed PSUM row + hi/lo rows
                    if skip_yprep:
                        return
                    slc = slice(jc * n_chunk, (jc + 1) * n_chunk)
                    psy = mp.tile([128, n_chunk], F32, tag="mm")
                    for j0 in range(0, n_chunk, mm_n):
                        nc.tensor.matmul(
                            psy[0:1, j0 : j0 + mm_n], mmi(nhi2[:, :]),
                            mmi(ysq[:, j0 : j0 + mm_n]),
                            start=True, stop=True,
                        )
                    nc.vector.tensor_copy(mmo(y_aug[64:65, slc]), psy[0:1, :])
                    if use_f32r:
                        # lo = exact - rounded hi (fp32r residual pair)
                        nc.vector.tensor_sub(
                            mmo(y_aug[96:97, slc]), psy[0:1, :], y_aug[64:65, slc],
                        )
                    else:
                        nc.vector.memset(y_aug[96:97, slc], 0.0)

                y_prep_post(0, y_prep_pre(0))

                # ---- bias transpose: one PSUM tile, 8 matmuls, one copy ----
                psb = mp.tile([128, n_chunk], F32, tag="mm")
                for i in range(n_tiles):
                    nc.tensor.matmul(
                        psb[:, i : i + 1], x2row[:, i * 128 : (i + 1) * 128],
                        ones11[:, :],
                        start=True, stop=True,
                    )
                nc.vector.tensor_copy(bias_sb[:, :], psb[:, 0:n_tiles])

                # ---- main sweep, column-major: per column chunk, all 8 row
                # tiles matmul+exp+ship, so early ACT work needs only early
                # y chunks. Chunk c+1's SBUF prep runs while ACT chews
                # chunk c; its PSUM half is emitted after chunk c's tiles. ----
                for c in range(mc):
                    ysq_next = y_prep_pre(c + 1) if c + 1 < mc else None
                    for i in range(n_tiles):
                        ps = mp.tile([128, n_chunk], F32, tag="mm")
                        if main_mm:
                            for _x in range(mm_x):
                                for jj in range(n_sub):
                                    col = c * n_chunk + jj * mm_n
                                    nc.tensor.matmul(
                                        ps[:, jj * mm_n : (jj + 1) * mm_n],
                                        mmi(x_aug[:, i * 128 : (i + 1) * 128]),
                                        mmi(y_aug[:, col : col + mm_n]),
                                        start=True, stop=True,
                                    )
                        else:
                            nc.vector.memset(ps[:, 0:1], 0.0)
                        ot = opool.tile([128, n_chunk], odt, tag="ot")
                        if act:
                            for _x in range(act_x):
                                nc.scalar.activation(
                                    ot[:, :], ps[:, :], AF.Exp,
                                    bias=bias_sb[:, i : i + 1],
                                )
                        else:
                            nc.vector.memset(ot[:, 0:1], 0.0)
                        if outdma:
                            rings[(c * n_tiles + i) % len(rings)].dma_start(
                                out=out[i * 128 : (i + 1) * 128,
                                        c * n_chunk : (c + 1) * n_chunk],
                                in_=ot[:, :],
                            )
                    if ysq_next is not None:
                        y_prep_post(c + 1, ysq_next)
    nc.finalize()
    return nc


_NC_CACHE = {}


def _get_nc():
    if "nc" not in _NC_CACHE:
        _NC_CACHE["nc"] = build_nc()
    return _NC_CACHE["nc"]


def stage_inputs(x, y, log_lengthscale, log_outputscale):
    x = np.ascontiguousarray(np.asarray(x, dtype=np.float32))
    y = np.ascontiguousarray(np.asarray(y, dtype=np.float32))
    lls = np.ascontiguousarray(np.asarray(log_lengthscale, dtype=np.float32))
    los = np.ascontiguousarray(np.asarray(log_outputscale, dtype=np.float32))

    yT = np.ascontiguousarray(y.T)  # [D, M]
    in_maps = []
    for c in range(N_CORES):
        xT_c = np.ascontiguousarray(x[c * NSH : (c + 1) * NSH].T)  # [D, NSH]
        in_maps.append(
            {"xT": xT_c, "yT": yT, "log_lengthscale": lls, "log_outputscale": los}
        )
    return in_maps


def assemble_output(out_concat):
    """Map the over-cores-concatenated device output to the final [N, M]."""
    return np.asarray(out_concat).astype(np.float32)


def kernel(x, y, log_lengthscale, log_outputscale):
    in_maps = stage_inputs(x, y, log_lengthscale, log_outputscale)
    res = run_bass_kernel_spmd(_get_nc(), in_maps, core_ids=list(range(N_CORES)))
    return assemble_output(np.concatenate([r["out"] for r in res.results], axis=0))


# revision 27
# speedup vs baseline: 1.0729x; 1.0729x over previous
"""ARD RBF kernel matrix on 8 TRN2 NeuronCores.

out[n, m] = exp(log_outputscale) * exp(-0.5 * sum_d ((x[n,d] - y[m,d]) / l_d)^2)
with l = exp(log_lengthscale).

Per core (rows of x sharded 8-ways), with invl2[d] = exp(-2*log_lengthscale[d]):
the lengthscale is folded into the X side only —
    cross[n, m] = sum_d (x[n,d]*invl2[d]) * y[m,d]
    y2[m] = sum_d y[m,d]^2 * (-0.5*invl2[d])   (matmul, weights -0.5*invl2)
    x2[n] = sum_d x[n,d]^2 * (-0.5*invl2[d]) + log_outputscale
    out[n, m] = Exp(cross + y2 + x2)
so y needs NO elementwise scaling: one DVE f32->f32r rounding copy and
one DVE square per chunk, and the y2 row rides the matmul. One K=97
fp32r matmul per output tile: lhsT = [x*invl2; 1; 0*31; 1],
rhs = [y; y2hi; junk*31; y2lo]. The zero rows in lhsT kill the junk
rows; y2 is split hi+lo (fp32r residual pair, both at DVE-writable
partitions 64/96) so its rounding error stays ~1e-6; x2 + log_os ride
the ACT bias in full f32. exp runs as one ScalarE pass per [128, 2048]
PSUM chunk, written as bf16 (8-bit exponent covers the e^-60-scale
tail; ~0.4% rounding) and upcast to f32 on the host — halves the
output HBM traffic. Measured HW rel err ~1e-3.

Inputs are staged host-side in transposed layout ([D, points]) so the
contraction dim lands on SBUF partitions with no on-device transposes.

Schedule: input DMAs first on the SP ring (x, then y in 4 chunks);
DVE preps x_aug while y chunk 0 lands; per y chunk DVE rounds + squares
and PE reduces y2 (borrowing row 0 of main-pool PSUM tiles — no
separate PSUM pool, no pool barrier); then the 8x16 main matmul /
exp sweep streams row tiles out as 1 MiB bf16 DMAs (first row tile
ships in 0.5 MiB chunks to start the output stream early).

build_nc(repeat=R) emits the whole computation R times into one NEFF
(reps serialized by buffer reuse) — used by test.py to measure the
per-iteration device makespan as a slope, amortizing dispatch overhead.
main_mm/act/outdma=False build ablation variants for differential
timing only (wrong results).
"""

import numpy as np

import concourse.bass as bass
import concourse.mybir as mybir
import concourse.tile as tile
from concourse import bacc
from concourse.bass_utils import run_bass_kernel_spmd

N_CORES = 8
N, M, D = 8192, 8192, 64
NSH = N // N_CORES  # 1024 x-rows per core

F32 = mybir.dt.float32
F32R = mybir.dt.float32r
BF16 = mybir.dt.bfloat16
AF = mybir.ActivationFunctionType


def build_nc(nsh=NSH, m=M, d=D, use_f32r=True, n_chunk=2048, out_bf16=True,
             repeat=1, main_mm=True, act=True, outdma=True, dma_rings=2,
             mm_x=1, act_x=1, skip_yprep=0):
    """Per-core Bass graph. SPMD: same graph on all 8 cores."""
    nc = bacc.Bacc("TRN2", target_bir_lowering=False)

    odt = BF16 if out_bf16 else F32

    xT = nc.dram_tensor("xT", [d, nsh], F32, kind="ExternalInput")
    yT = nc.dram_tensor("yT", [d, m], F32, kind="ExternalInput")
    lls = nc.dram_tensor("log_lengthscale", [d], F32, kind="ExternalInput")
    los = nc.dram_tensor("log_outputscale", [1], F32, kind="ExternalInput")
    out = nc.dram_tensor("out", [nsh, m], odt, kind="ExternalOutput")

    n_tiles = nsh // 128          # x tiles (output partition dim)
    mm_n = 512                    # moving free dim per matmul (one PSUM bank)
    n_sub = n_chunk // mm_n       # matmuls per ACT chunk
    mc = m // n_chunk             # y chunks
    K = 65                        # contraction: 64 data rows + centered-y2 row

    def mmi(ap):  # matmul input view
        return ap.bitcast(F32R) if use_f32r else ap

    def mmo(ap):  # rounded-writer output view (BIR fp32r-producer rule)
        return ap.bitcast(F32R) if use_f32r else ap

    # output DMA queue rotation (SP always; optionally ACT HWDGE / SWDGE)
    rings = [nc.sync, nc.scalar, nc.gpsimd][:dma_rings]

    with tile.TileContext(nc) as tc:
        with (
            tc.tile_pool(name="const", bufs=1) as cpool,
            tc.tile_pool(name="xb", bufs=2) as xb,
            tc.tile_pool(name="yraw", bufs=2) as yrp,
            tc.tile_pool(name="ysqp", bufs=2) as ysp,
            tc.tile_pool(name="outp", bufs=4) as opool,
            tc.tile_pool(name="mainps", bufs=2, space="PSUM") as mp,
        ):
            # ---- hyperparameters and constants (once; reps reuse) ----
            lls_sb = cpool.tile([d, 1], F32, tag="lls")
            nc.sync.dma_start(out=lls_sb[:, :], in_=lls[:].rearrange("(d o) -> d o", o=1))
            los_sb = cpool.tile([1, 1], F32, tag="los")
            nc.sync.dma_start(out=los_sb[:, :], in_=los[:].rearrange("(a o) -> a o", o=1))

            invl2 = cpool.tile([d, 1], F32, tag="invl2")  # exp(-2*lls)
            nc.scalar.activation(invl2[:, :], lls_sb[:, :], AF.Exp, scale=-2.0)
            l2 = cpool.tile([d, 1], F32, tag="l2")  # exp(+2*lls)
            nc.scalar.activation(l2[:, :], lls_sb[:, :], AF.Exp, scale=2.0)
            neghalf_f = cpool.tile([d, 1], F32, tag="neghalf_f")
            nc.vector.memset(neghalf_f[:, :], -0.5)
            # y2 reduce weights -0.5*invl2 (ysq holds rounded raw y squares);
            # x2 reduce weights -0.5*exp(2*lls) (xsq holds rounded x*invl2
            # squares) — together they keep sq_dist a perfect square:
            # c + x2 + y2 = -0.5*sum((a/l - y*invl)^2) for a = (x*invl2)_r
            nhi2 = cpool.tile([d, 1], F32, tag="nhi2")
            nc.vector.tensor_mul(mmo(nhi2[:, :]), invl2[:, :], neghalf_f[:, :])
            nhx2 = cpool.tile([d, 1], F32, tag="nhx2")
            nc.vector.tensor_mul(mmo(nhx2[:, :]), l2[:, :], neghalf_f[:, :])
            ones11 = cpool.tile([1, 1], F32, tag="ones11")
            nc.vector.memset(ones11[:, :], 1.0)

            # lhsT aux row: ones at row 64 (pairs with the y2 row of rhs)
            ones_row = cpool.tile([1, nsh], F32, tag="ones_row")
            nc.vector.memset(ones_row[:, :], 1.0)

            # y2 centering constant C = 0.5*sum_d invl2_d (~= E[-y2] for
            # unit-variance y): the stored y2 row is (y2 + C)_f32r, centered
            # near 0 so its relative ulp stays small (~5x less rounding
            # error than storing y2 raw, and no lo-residual row needed);
            # -C is folded into the ACT bias below.
            half_col = cpool.tile([d, 1], F32, tag="half_col")
            nc.vector.memset(half_col[:, :], 0.5)
            psc = mp.tile([128, n_chunk], F32, tag="mm")
            nc.tensor.matmul(psc[0:1, 0:1], invl2[:, :], half_col[:, :],
                             start=True, stop=True)
            cneg = cpool.tile([1, 1], F32, tag="cneg")  # los - C
            nc.vector.tensor_sub(cneg[:, :], los_sb[:, :], psc[0:1, 0:1])
            c_sb = cpool.tile([1, 1], F32, tag="c_sb")
            nc.vector.tensor_copy(c_sb[:, :], psc[0:1, 0:1])

            x2row = cpool.tile([1, nsh], F32, tag="x2row")
            bias_sb = cpool.tile([128, n_tiles], F32, tag="bias")
            y_aug = cpool.tile([K, m], F32, tag="y_aug")

            for _rep in range(repeat):
                # ---- input DMAs on the SWDGE (gpsimd) queue: never queue
                # behind the previous rep's output DMAs on the SP ring ----
                x_raw = xb.tile([d, nsh], F32, tag="x_raw")
                nc.gpsimd.dma_start(out=x_raw[:, :], in_=xT[:, :])
                y_raws = []
                for jc in range(mc):
                    y_raw = yrp.tile([d, n_chunk], F32, tag="y_raw")
                    nc.gpsimd.dma_start(
                        out=y_raw[:, :], in_=yT[:, jc * n_chunk : (jc + 1) * n_chunk]
                    )
                    y_raws.append(y_raw)

                # ---- x side (DVE): x_aug = [x*invl2; 1; 0...; 1], xsq = x^2 ----
                x_aug = xb.tile([K, nsh], F32, tag="x_aug")
                nc.vector.tensor_scalar_mul(mmo(x_aug[0:d, :]), x_raw[:, :], invl2[:, :])
                # square the ROUNDED operands (not raw) so sq_dist keeps its
                # perfect-square structure in the rounded values
                xsq = xb.tile([d, nsh], F32, tag="xsq")
                nc.vector.tensor_mul(mmo(xsq[:, :]), x_aug[0:d, :], x_aug[0:d, :])
                nc.vector.tensor_copy(mmo(x_aug[64:K, :]), ones_row[:, :])

                # ---- x2 row: -0.5*sum x^2*exp(2lls) + log_os (one PSUM tile,
                # one ACT identity) ----
                psx = mp.tile([128, n_chunk], F32, tag="mm")
                for j0 in range(0, nsh, mm_n):
                    nc.tensor.matmul(
                        psx[0:1, j0 : j0 + mm_n], mmi(nhx2[:, :]),
                        mmi(xsq[:, j0 : j0 + mm_n]),
                        start=True, stop=True,
                    )
                nc.scalar.activation(
                    x2row[:, :], psx[0:1, 0:nsh], AF.Identity, bias=cneg[:, :],
                )

                # ---- y prep, split so the PSUM-borrowing half (psy matmuls
                # + hi/lo rows) can be emitted AFTER a main chunk's 8 tiles:
                # its borrowed buffer then has a whole ACT-chunk window
                # (~14us) before the next main tile WARs on it. ----
                # skip_yprep ablation levels: 1 = skip hi/lo rows,
                # 2 = also skip psy matmuls, 3 = skip everything
                def y_prep_pre(jc):
                    # rounding copy + square: SBUF only, no PSUM deps
                    if skip_yprep >= 3:
                        return None
                    slc = slice(jc * n_chunk, (jc + 1) * n_chunk)
                    y_raw = y_raws[jc]
                    nc.vector.tensor_copy(mmo(y_aug[0:d, slc]), y_raw[:, :])
                    ysq = ysp.tile([d, n_chunk], F32, tag="ysq")
                    nc.vector.tensor_mul(
                        mmo(ysq[:, :]), y_aug[0:d, slc], y_aug[0:d, slc]
                    )
                    return ysq

                def y_prep_post(jc, ysq):
                    # y2 reduce into a borrowed PSUM row, then the single
                    # centered y2 row: (y2 + C)_f32r
                    if skip_yprep >= 2:
                        return
                    slc = slice(jc * n_chunk, (jc + 1) * n_chunk)
                    psy = mp.tile([128, n_chunk], F32, tag="mm")
                    for j0 in range(0, n_chunk, mm_n):
                        nc.tensor.matmul(
                            psy[0:1, j0 : j0 + mm_n], mmi(nhi2[:, :]),
                            mmi(ysq[:, j0 : j0 + mm_n]),
                            start=True, stop=True,
                        )
                    if skip_yprep >= 1:
                        return
                    nc.vector.tensor_scalar_add(
                        mmo(y_aug[64:65, slc]), psy[0:1, :], c_sb[:, :]
                    )

                y_prep_post(0, y_prep_pre(0))

                # ---- bias transpose: one PSUM tile, 8 matmuls, one copy ----
                psb = mp.tile([128, n_chunk], F32, tag="mm")
                for i in range(n_tiles):
                    nc.tensor.matmul(
                        psb[:, i : i + 1], x2row[:, i * 128 : (i + 1) * 128],
                        ones11[:, :],
                        start=True, stop=True,
                    )
                nc.vector.tensor_copy(bias_sb[:, :], psb[:, 0:n_tiles])

                # ---- main sweep, column-major: per column chunk, all 8 row
                # tiles matmul+exp+ship, so early ACT work needs only early
                # y chunks. Chunk c+1's SBUF prep runs while ACT chews
                # chunk c; its PSUM half is emitted after chunk c's tiles. ----
                for c in range(mc):
                    ysq_next = y_prep_pre(c + 1) if c + 1 < mc else None
                    for i in range(n_tiles):
                        ps = mp.tile([128, n_chunk], F32, tag="mm")
                        if main_mm:
                            for _x in range(mm_x):
                                for jj in range(n_sub):
                                    col = c * n_chunk + jj * mm_n
                                    nc.tensor.matmul(
                                        ps[:, jj * mm_n : (jj + 1) * mm_n],
                                        mmi(x_aug[:, i * 128 : (i + 1) * 128]),
                                        mmi(y_aug[:, col : col + mm_n]),
                                        start=True, stop=True,
                                    )
                        else:
                            nc.vector.memset(ps[:, 0:1], 0.0)
                        ot = opool.tile([128, n_chunk], odt, tag="ot")
                        if act:
                            for _x in range(act_x):
                                nc.scalar.activation(
                                    ot[:, :], ps[:, :], AF.Exp,
                                    bias=bias_sb[:, i : i + 1],
                                )
                        else:
                            nc.vector.memset(ot[:, 0:1], 0.0)
                        if outdma:
                            rings[(c * n_tiles + i) % len(rings)].dma_start(
                                out=out[i * 128 : (i + 1) * 128,
                                        c * n_chunk : (c + 1) * n_chunk],
                                in_=ot[:, :],
                            )
                    if ysq_next is not None:
                        y_prep_post(c + 1, ysq_next)
    nc.finalize()
    return nc


_NC_CACHE = {}


def _get_nc():
    if "nc" not in _NC_CACHE:
        _NC_CACHE["nc"] = build_nc()
    return _NC_CACHE["nc"]


def stage_inputs(x, y, log_lengthscale, log_outputscale):
    x = np.ascontiguousarray(np.asarray(x, dtype=np.float32))
    y = np.ascontiguousarray(np.asarray(y, dtype=np.float32))
    lls = np.ascontiguousarray(np.asarray(log_lengthscale, dtype=np.float32))
    los = np.ascontiguousarray(np.asarray(log_outputscale, dtype=np.float32))

    yT = np.ascontiguousarray(y.T)  # [D, M]
    in_maps = []
    for c in range(N_CORES):
        xT_c = np.ascontiguousarray(x[c * NSH : (c + 1) * NSH].T)  # [D, NSH]
        in_maps.append(
            {"xT": xT_c, "yT": yT, "log_lengthscale": lls, "log_outputscale": los}
        )
    return in_maps


def assemble_output(out_concat):
    """Map the over-cores-concatenated device output to the final [N, M]."""
    return np.asarray(out_concat).astype(np.float32)


def kernel(x, y, log_lengthscale, log_outputscale):
    in_maps = stage_inputs(x, y, log_lengthscale, log_outputscale)
    res = run_bass_kernel_spmd(_get_nc(), in_maps, core_ids=list(range(N_CORES)))
    return assemble_output(np.concatenate([r["out"] for r in res.results], axis=0))
